# revision 1
# baseline (speedup 1.0000x reference)
"""Trainium2 Bass kernel for nn_ContrastiveLoss (B=4, C=256, H=W=256).

Strategy
--------
The reference computes four families of per-position channel dot products
over columns of x viewed as [B, C, N] (N = H*W), then scalar reductions:

  fam1 (pos_sim): dot(x[:,:,pos[t]],  x[:,:,pos[t+P]])   t in [0,P)
  fam2 (neg_sim): dot(x[:,:,neg[t]],  x[:,:,neg[t+Ng]])  t in [0,Ng)
  fam3 (pn1):     dot(x[:,:,pos[t]],  x[:,:,neg[t]])     t in [0,M)
  fam4 (pn2):     dot(x[:,:,pos[t]],  x[:,:,neg[t]])     t in [M,2M)

Each column of x participates in at most two dot products, so the union of
the four pairings is a degree-<=2 graph = disjoint paths and even cycles.
The host walks those paths/cycles and emits columns in walk order; in the
permuted tensor xp every dot product is between ADJACENT columns.  The
device then only streams xp once (the HBM roofline), computes shifted
products xp[:,:,i]*xp[:,:,i+1], reduces over C via a ones-vector matmul on
the tensor engine (PSUM-accumulated over the two 128-channel chunks), and
applies per-family 0/1 masks to form the four partial reductions.  Cycles
are closed by re-emitting their first column; junk edges between components
and in padding have all-zero masks.

Sharding: the edge list is split into 8 equal contiguous chunks of the
column walk (one per NeuronCore, overlapping by one column).  Each core
returns 4 partial scalars (sum d*m1, sum d*m2, sum exp(d)*m3, sum
exp(d)*m4); the host combines them into the final loss.  exp() needs no
max-subtraction: |d| < ~0.5 for this data regime, so sum(exp(d)) is stable
in fp32 (guarded by an assert on the host).
"""

import math
import sys

import numpy as np

if "/opt/trn_rl_repo" not in sys.path:  # harness runs from a fresh dir
    sys.path.insert(0, "/opt/trn_rl_repo")

B, C, N = 4, 256, 65536
N_CORES = 8
BLOCK = 512          # edges per PSUM block (= max fp32 matmul free dim)
CHUNKS = C // 128    # channel chunks of 128 partitions


# ---------------------------------------------------------------- host prep

def _build_walk(y):
    """Column permutation + per-edge family labels (0 = junk/padding)."""
    y = np.asarray(y).reshape(-1)
    pos_idx = np.nonzero(y == 1)[0]
    neg_idx = np.nonzero(y == 0)[0]
    P = pos_idx.shape[0] // 2
    Ng = neg_idx.shape[0] // 2
    M = min(P, Ng)

    nP, nN = 2 * P, 2 * Ng
    V = nP + nN
    t_pos = np.arange(nP)
    t_neg = np.arange(nN)
    nbrA = np.empty(V, dtype=np.int64)
    nbrA[:nP] = np.where(t_pos < P, t_pos + P, t_pos - P)
    nbrA[nP:] = nP + np.where(t_neg < Ng, t_neg + Ng, t_neg - Ng)
    famA = np.empty(V, dtype=np.int8)
    famA[:nP] = 1
    famA[nP:] = 2
    nbrB = np.full(V, -1, dtype=np.int64)
    nbrB[:2 * M] = nP + t_pos[:2 * M]
    nbrB[nP:nP + 2 * M] = t_neg[:2 * M]
    famB = np.zeros(V, dtype=np.int8)
    famB[:M] = 3
    famB[M:2 * M] = 4
    famB[nP:nP + M] = 3
    famB[nP + M:nP + 2 * M] = 4

    visited = np.zeros(V, dtype=bool)
    order = np.empty(V + V // 4 + 16, dtype=np.int64)
    fams_l = np.empty(order.shape[0], dtype=np.int8)
    no = 0
    ne = 0

    def walk_from(v0, is_cycle):
        nonlocal no, ne
        if no > 0:
            fams_l[ne] = 0  # junk edge joining the previous component
            ne += 1
        v = v0
        use_A = True  # endpoints/cycle starts leave via their A edge
        order[no] = v
        no += 1
        visited[v] = True
        while True:
            if use_A:
                nxt, fam = nbrA[v], famA[v]
            else:
                nxt = nbrB[v]
                if nxt < 0:
                    return
                fam = famB[v]
            if visited[nxt]:
                if is_cycle and nxt == v0 and not use_A:
                    fams_l[ne] = fam
                    ne += 1
                    order[no] = v0  # close the cycle
                    no += 1
                return
            fams_l[ne] = fam
            ne += 1
            order[no] = nxt
            no += 1
            visited[nxt] = True
            v = nxt
            use_A = not use_A

    for v0 in np.nonzero(nbrB < 0)[0]:
        if not visited[v0]:
            walk_from(int(v0), is_cycle=False)
    for v0 in range(V):
        if not visited[v0]:
            walk_from(int(v0), is_cycle=True)

    n_real = int((fams_l[:ne] > 0).sum())
    assert n_real == P + Ng + 2 * M, (n_real, P + Ng + 2 * M)

    per = N_CORES * BLOCK
    E_pad = ((ne + per - 1) // per) * per
    V_pad = E_pad + 1
    fams = np.zeros(E_pad, dtype=np.int8)
    fams[:ne] = fams_l[:ne]
    vert = np.zeros(V_pad, dtype=np.int64)
    vert[:no] = order[:no]
    colmap = np.where(vert < nP, pos_idx[np.minimum(vert, nP - 1)],
                      neg_idx[np.maximum(vert - nP, 0)])
    return colmap, fams, P, Ng, M


# ------------------------------------------------------------- device program

def trace_program(nc, tc, ctx, S, nb, dt_in, **prog_opts):
    """Emit the per-core program. S = edges/core, nb = S//BLOCK.

    DRAM tensors (per core): xp [B, C, S+1] dt_in, msk [4, 4*nb, BLOCK] f32,
    out [1, 4] f32 = (sum d*m1, sum d*m2, sum exp(d)*m3, sum exp(d)*m4).
    """
    import concourse.mybir as mybir

    f32 = mybir.dt.float32
    R = 4 * nb
    xp = nc.dram_tensor("xp", [B, C, S + 1], dt_in, kind="ExternalInput").ap()
    msk = nc.dram_tensor("msk", [4, R, BLOCK], f32, kind="ExternalInput").ap()
    out = nc.dram_tensor("out", [1, 4], f32, kind="ExternalOutput").ap()
    trace_program_aps(nc, tc, ctx, S, nb, dt_in, xp, msk, out, **prog_opts)


def trace_program_aps(nc, tc, ctx, S, nb, dt_in, xp, msk, out,
                      gpsimd_on=(2, 5, 7), xp_bufs=10, prod_bufs=6,
                      kb_blocks=None, psum_chains=1):
    import concourse.mybir as mybir

    f32 = mybir.dt.float32
    R = 4 * nb  # d rows: block k, batch b -> row 4k+b

    const_pool = ctx.enter_context(tc.tile_pool(name="const", bufs=1))
    mask_pool = ctx.enter_context(tc.tile_pool(name="masks", bufs=1))
    xp_pool = ctx.enter_context(tc.tile_pool(name="xp", bufs=xp_bufs))
    prod_pool = ctx.enter_context(tc.tile_pool(name="prod", bufs=prod_bufs))
    stat_pool = ctx.enter_context(tc.tile_pool(name="stat", bufs=1))
    psum_pool = ctx.enter_context(tc.tile_pool(name="psum", bufs=1, space="PSUM"))

    # Staircase selector: zo[:, 63] = 1, else 0.  lhsT = zo[:, 63-r : 127-r]
    # is a [128, R] one-hot-column matrix that routes a partition-dim
    # column-sum into PSUM row r (other rows accumulate exact zeros) --
    # matmul PSUM outputs must start at partition 0/32/64, so rows can't be
    # addressed via the output AP.
    dt_prod = dt_in  # DVE converts on write for free; PE runs 16-bit at rate
    zo = const_pool.tile([128, 63 + R], dt_prod)
    nc.vector.memset(zo[:], 0.0)
    nc.vector.memset(zo[:, 63:64], 1.0)
    ones_f32 = const_pool.tile([128, 1], f32)
    nc.vector.memset(ones_f32[:], 1.0)

    m_tiles = []
    for f in range(4):
        mt = mask_pool.tile([R, BLOCK], f32, tag=f"m{f}")
        nc.sync.dma_start(mt[:], msk[f])
        m_tiles.append(mt)

    d_psums = [psum_pool.tile([R, BLOCK], f32, tag=f"d{i}", name=f"d_psum{i}")
               for i in range(psum_chains)]

    # KB 512-edge blocks per DMA/mul tile: fewer, larger DMAs and DVE ops
    KB = kb_blocks or (4 if nb % 4 == 0 else (2 if nb % 2 == 0 else 1))
    assert nb % KB == 0
    W = KB * BLOCK
    n_mm = nb * B * CHUNKS
    mm_per_chain = n_mm // psum_chains
    assert n_mm % psum_chains == 0
    chain_cnt = [0] * psum_chains
    i_mm = 0
    mul_i = 0
    for kb in range(nb // KB):
        for b in range(B):
            prods = []
            for c in range(CHUNKS):
                t = xp_pool.tile([128, W + 1], dt_in)
                nc.sync.dma_start(
                    t[:], xp[b, 128 * c:128 * (c + 1), W * kb:W * (kb + 1) + 1])
                p = prod_pool.tile([128, W], dt_prod)
                # DVE is the mul bottleneck; GpSimd runs these ~1.7x
                # slower, so balance ~3/8 of them onto it
                eng = nc.gpsimd if (gpsimd_on and
                                    mul_i % 8 in gpsimd_on) else nc.vector
                eng.tensor_mul(p[:], t[:, 0:W], t[:, 1:W + 1])
                mul_i += 1
                prods.append(p)
            for j in range(KB):
                row = 4 * (kb * KB + j) + b
                for c in range(CHUNKS):
                    ch = i_mm % psum_chains
                    nc.tensor.matmul(
                        d_psums[ch][:, :], zo[:, 63 - row:63 - row + R],
                        prods[c][:, BLOCK * j:BLOCK * (j + 1)],
                        start=(chain_cnt[ch] == 0),
                        stop=(chain_cnt[ch] == mm_per_chain - 1))
                    chain_cnt[ch] += 1
                    i_mm += 1

    if psum_chains == 1:
        d_fin = d_psums[0]
    else:
        d_fin = stat_pool.tile([R, BLOCK], f32, tag="dfin")
        nc.scalar.copy(d_fin[:], d_psums[0][:])
        for i in range(1, psum_chains):
            nc.vector.tensor_add(d_fin[:], d_fin[:], d_psums[i][:])
    exp_sb = stat_pool.tile([R, BLOCK], f32)
    nc.scalar.activation(exp_sb[:], d_fin[:],
                         mybir.ActivationFunctionType.Exp)

    rcat = stat_pool.tile([R, 4], f32)
    srcs = [d_fin, d_fin, exp_sb, exp_sb]
    for f in range(4):
        scratch = stat_pool.tile([R, BLOCK], f32, tag="scr")
        nc.vector.tensor_mul(scratch[:], srcs[f][:], m_tiles[f][:])
        nc.vector.reduce_sum(rcat[:, f:f + 1], scratch[:],
                             axis=mybir.AxisListType.X)

    f_psum = psum_pool.tile([1, 4], f32, tag="final")
    nc.tensor.matmul(f_psum[:], ones_f32[0:R, :], rcat[:], start=True, stop=True)
    res = stat_pool.tile([1, 4], f32)
    nc.scalar.copy(res[:], f_psum[:])
    nc.sync.dma_start(out, res[:])


_CACHE = {}


def _compiled(S, nb, dt_name, prog_opts=None):
    key = (S, nb, dt_name, tuple(sorted((prog_opts or {}).items(),
                                        key=lambda kv: kv[0])))
    if key in _CACHE:
        return _CACHE[key]
    from contextlib import ExitStack

    import concourse.bacc as bacc
    import concourse.mybir as mybir
    import concourse.tile as tile

    dt_in = getattr(mybir.dt, dt_name)
    nc = bacc.Bacc("TRN2", target_bir_lowering=False, debug=False,
                   num_devices=N_CORES)
    with tile.TileContext(nc) as tc:
        with ExitStack() as ctx:
            trace_program(nc, tc, ctx, S, nb, dt_in, **(prog_opts or {}))
    nc.compile()
    _CACHE[key] = nc
    return nc


# -------------------------------------------------------------------- kernel

def kernel(x, y, _dt_name="float16", _run_opts=None, _prog_opts=None):
    x = np.asarray(x)
    y = np.asarray(y)
    assert x.shape == (B, C, 256, 256) and y.shape == (N,)

    colmap, fams, P, Ng, M = _build_walk(y)
    E = fams.shape[0]
    S = E // N_CORES
    nb = S // BLOCK
    assert nb * BLOCK * N_CORES == E and 4 * nb <= 128

    np_dt = {"float32": np.float32, "float16": np.float16}[_dt_name]
    x3 = x.reshape(B, C, N)
    xp = np.ascontiguousarray(x3[:, :, colmap], dtype=np_dt)  # [B, C, E+1]

    # masks in d-row layout: row 4k+b of core i covers edges
    # i*S + k*BLOCK + [0, BLOCK), identical for the 4 b rows
    fams_c = fams.reshape(N_CORES, nb, 1, BLOCK)
    m = np.empty((N_CORES, 4, 4 * nb, BLOCK), dtype=np.float32)
    for f in range(4):
        m[:, f] = np.broadcast_to(fams_c == f + 1,
                                  (N_CORES, nb, 4, BLOCK)
                                  ).reshape(N_CORES, 4 * nb, BLOCK)

    in_maps = [
        {"xp": np.ascontiguousarray(xp[:, :, i * S:(i + 1) * S + 1]),
         "msk": m[i]}
        for i in range(N_CORES)
    ]

    nc = _compiled(S, nb, _dt_name, _prog_opts)
    from concourse.bass_utils import run_bass_kernel_spmd

    res = run_bass_kernel_spmd(nc, in_maps, list(range(N_CORES)),
                               **(_run_opts or {}))
    partials = np.stack([r["out"][0] for r in res.results])  # [N_CORES, 4]
    s1, s2, s3, s4 = partials.sum(axis=0, dtype=np.float64)

    n = float(B * M)
    loss = (-s1 / (B * P) - s2 / (B * Ng)
            + math.log(s3) - math.log(n) + math.log(s4) - math.log(n))
    assert np.isfinite(loss)
    out = np.float32(loss)
    if _run_opts:
        return out, res
    return out



# revision 5
# speedup vs baseline: 1.0879x; 1.0879x over previous
"""Trainium2 Bass kernel for nn_ContrastiveLoss (B=4, C=256, H=W=256).

Strategy
--------
The reference computes four families of per-position channel dot products
over columns of x viewed as [B, C, N] (N = H*W), then scalar reductions:

  fam1 (pos_sim): dot(x[:,:,pos[t]],  x[:,:,pos[t+P]])   t in [0,P)
  fam2 (neg_sim): dot(x[:,:,neg[t]],  x[:,:,neg[t+Ng]])  t in [0,Ng)
  fam3 (pn1):     dot(x[:,:,pos[t]],  x[:,:,neg[t]])     t in [0,M)
  fam4 (pn2):     dot(x[:,:,pos[t]],  x[:,:,neg[t]])     t in [M,2M)

Each column of x participates in at most two dot products, so the union of
the four pairings is a degree-<=2 graph = disjoint paths and even cycles.
The host walks those paths/cycles and emits columns in walk order; in the
permuted tensor xp every dot product is between ADJACENT columns.  The
device then only streams xp once (the HBM roofline), computes shifted
products xp[:,:,i]*xp[:,:,i+1] on the DVE (fp16 2x mode; GpSimd is NOT
used -- concurrent GpSimd muls drop the DVE out of 2x mode via SBUF port
contention, a large net loss), reduces over C via a ones-staircase matmul
on the tensor engine (PSUM-accumulated), and applies per-family 0/1 masks
(fused multiply+reduce on DVE) to form the four partial reductions.

Device balance: the PE staircase costs ~1 matmul per 512 edges per 128-ch
chunk; measured ~480ns each incl. LDWEIGHTS, so a pure 2-matmul-per-block
program is PE-bound.  For the largest tiles (half of all data) the two
128-channel product chunks are first combined with a DVE add, halving the
matmul count for those blocks.  This balances DVE (~46us) against PE
(~46us) under the ~48us fp16 DMA roofline.

DMA startup: tiles are streamed in a small-to-large width schedule
(512,512,1024,2048,4096 columns) so the first products are available a few
us after launch instead of ~16us (the SDMA engines round-robin all queued
transfers, so the first tile's latency scales with total queued bytes).

Sharding: the edge list is split into 8 equal contiguous chunks of the
column walk (one per NeuronCore, overlapping by one column).  Each core
returns 4 partial scalars; the host combines them into the final loss.
exp() needs no max-subtraction: |d| < ~0.5 for this data regime.
"""

import math
import sys

import numpy as np

if "/opt/trn_rl_repo" not in sys.path:  # harness runs from a fresh dir
    sys.path.insert(0, "/opt/trn_rl_repo")

B, C, N = 4, 256, 65536
N_CORES = 8
BLOCK = 512          # edges per PSUM block (= max fp32 matmul free dim)
CHUNKS = C // 128    # channel chunks of 128 partitions


# ---------------------------------------------------------------- host prep

def _build_walk(y):
    """Column permutation + per-edge family labels (0 = junk/padding)."""
    y = np.asarray(y).reshape(-1)
    pos_idx = np.nonzero(y == 1)[0]
    neg_idx = np.nonzero(y == 0)[0]
    P = pos_idx.shape[0] // 2
    Ng = neg_idx.shape[0] // 2
    M = min(P, Ng)

    nP, nN = 2 * P, 2 * Ng
    V = nP + nN
    t_pos = np.arange(nP)
    t_neg = np.arange(nN)
    nbrA = np.empty(V, dtype=np.int64)
    nbrA[:nP] = np.where(t_pos < P, t_pos + P, t_pos - P)
    nbrA[nP:] = nP + np.where(t_neg < Ng, t_neg + Ng, t_neg - Ng)
    famA = np.empty(V, dtype=np.int8)
    famA[:nP] = 1
    famA[nP:] = 2
    nbrB = np.full(V, -1, dtype=np.int64)
    nbrB[:2 * M] = nP + t_pos[:2 * M]
    nbrB[nP:nP + 2 * M] = t_neg[:2 * M]
    famB = np.zeros(V, dtype=np.int8)
    famB[:M] = 3
    famB[M:2 * M] = 4
    famB[nP:nP + M] = 3
    famB[nP + M:nP + 2 * M] = 4

    visited = np.zeros(V, dtype=bool)
    order = np.empty(V + V // 4 + 16, dtype=np.int64)
    fams_l = np.empty(order.shape[0], dtype=np.int8)
    no = 0
    ne = 0

    def walk_from(v0, is_cycle):
        nonlocal no, ne
        if no > 0:
            fams_l[ne] = 0  # junk edge joining the previous component
            ne += 1
        v = v0
        use_A = True  # endpoints/cycle starts leave via their A edge
        order[no] = v
        no += 1
        visited[v] = True
        while True:
            if use_A:
                nxt, fam = nbrA[v], famA[v]
            else:
                nxt = nbrB[v]
                if nxt < 0:
                    return
                fam = famB[v]
            if visited[nxt]:
                if is_cycle and nxt == v0 and not use_A:
                    fams_l[ne] = fam
                    ne += 1
                    order[no] = v0  # close the cycle
                    no += 1
                return
            fams_l[ne] = fam
            ne += 1
            order[no] = nxt
            no += 1
            visited[nxt] = True
            v = nxt
            use_A = not use_A

    for v0 in np.nonzero(nbrB < 0)[0]:
        if not visited[v0]:
            walk_from(int(v0), is_cycle=False)
    for v0 in range(V):
        if not visited[v0]:
            walk_from(int(v0), is_cycle=True)

    n_real = int((fams_l[:ne] > 0).sum())
    assert n_real == P + Ng + 2 * M, (n_real, P + Ng + 2 * M)

    per = N_CORES * BLOCK
    E_pad = ((ne + per - 1) // per) * per
    V_pad = E_pad + 1
    fams = np.zeros(E_pad, dtype=np.int8)
    fams[:ne] = fams_l[:ne]
    vert = np.zeros(V_pad, dtype=np.int64)
    vert[:no] = order[:no]
    colmap = np.where(vert < nP, pos_idx[np.minimum(vert, nP - 1)],
                      neg_idx[np.maximum(vert - nP, 0)])
    return colmap, fams, P, Ng, M


# ------------------------------------------------------------- device program

def _schedule(S, sched=None):
    """Per-(b,chunk) DMA tile widths, small-to-large for fast pipe start."""
    if sched is None:
        sched = [512, 512, 1024, 2048]
        rem = S - sum(sched)
        assert rem >= 0 and rem % 4096 == 0
        sched += [4096] * (rem // 4096)
    assert sum(sched) == S and all(w % 512 == 0 for w in sched)
    return sched


def trace_program(nc, tc, ctx, S, nb, dt_in, **prog_opts):
    """Emit the per-core program. S = edges/core, nb = S//BLOCK.

    DRAM tensors (per core): xp [B, C, S+1] dt_in, msk [4, 4*nb, BLOCK] f16,
    out [1, 4] f32 = (sum d*m1, sum d*m2, sum exp(d)*m3, sum exp(d)*m4).
    """
    import concourse.mybir as mybir

    f32 = mybir.dt.float32
    f16 = mybir.dt.float16
    R = 4 * nb
    xp = nc.dram_tensor("xp", [B, C, S + 1], dt_in, kind="ExternalInput").ap()
    msk = nc.dram_tensor("msk", [4, R, BLOCK], f16, kind="ExternalInput").ap()
    out = nc.dram_tensor("out", [1, 4], f32, kind="ExternalOutput").ap()
    trace_program_aps(nc, tc, ctx, S, nb, dt_in, xp, msk, out, **prog_opts)


def trace_program_aps(nc, tc, ctx, S, nb, dt_in, xp, msk, out,
                      xp_bufs=10, prod_bufs=6, sched=None, combine_min=4096):
    import concourse.mybir as mybir

    f32 = mybir.dt.float32
    R = 4 * nb  # d rows: block k, batch b -> row 4k+b
    widths = _schedule(S, sched)

    const_pool = ctx.enter_context(tc.tile_pool(name="const", bufs=1))
    mask_pool = ctx.enter_context(tc.tile_pool(name="masks", bufs=1))
    xp_pool = ctx.enter_context(tc.tile_pool(name="xp", bufs=xp_bufs))
    prod_pool = ctx.enter_context(tc.tile_pool(name="prod", bufs=prod_bufs))
    stat_pool = ctx.enter_context(tc.tile_pool(name="stat", bufs=1))
    psum_pool = ctx.enter_context(tc.tile_pool(name="psum", bufs=1, space="PSUM"))

    # Staircase selector: zo[:, 63] = 1, else 0.  lhsT = zo[:, 63-r : 63-r+R]
    # is a [128, R] one-hot-column matrix that routes a partition-dim
    # column-sum into PSUM row r (other rows accumulate exact zeros) --
    # matmul PSUM outputs must start at partition 0/32/64, so rows can't be
    # addressed via the output AP.
    zo = const_pool.tile([128, 63 + R], dt_in)
    nc.vector.memset(zo[:], 0.0)
    nc.vector.memset(zo[:, 63:64], 1.0)
    ones_f32 = const_pool.tile([128, 1], f32)
    nc.vector.memset(ones_f32[:], 1.0)

    d_psum = psum_pool.tile([R, BLOCK], f32, name="d_psum")

    n_mm = 0
    for w in widths:
        mm_per_row = 1 if w >= combine_min else CHUNKS
        n_mm += (w // BLOCK) * mm_per_row * B

    i_mm = 0
    blk = 0
    off = 0
    masks_sent = False
    for w in widths:
        combine = w >= combine_min
        for b in range(B):
            prods = []
            for c in range(CHUNKS):
                t = xp_pool.tile([128, w + 1], dt_in, tag="xp")
                nc.sync.dma_start(
                    t[:, 0:w + 1],
                    xp[b, 128 * c:128 * (c + 1), off:off + w + 1])
                p = prod_pool.tile([128, w], dt_in, tag="prod")
                nc.vector.tensor_mul(p[:, 0:w], t[:, 0:w], t[:, 1:w + 1])
                prods.append(p)
            if combine:
                nc.vector.tensor_add(prods[0][:, 0:w], prods[0][:, 0:w],
                                     prods[1][:, 0:w])
                prods = [prods[0]]
            for j in range(w // BLOCK):
                row = 4 * (blk + j) + b
                for p in prods:
                    nc.tensor.matmul(
                        d_psum[:, :], zo[:, 63 - row:63 - row + R],
                        p[:, BLOCK * j:BLOCK * (j + 1)],
                        start=(i_mm == 0), stop=(i_mm == n_mm - 1))
                    i_mm += 1
        blk += w // BLOCK
        off += w
        if not masks_sent:  # after the first (smallest) wave of xp tiles
            m_tiles = []
            for f in range(4):
                mt = mask_pool.tile([R, BLOCK], mybir.dt.float16, tag=f"m{f}")
                nc.sync.dma_start(mt[:], msk[f])
                m_tiles.append(mt)
            masks_sent = True
    assert i_mm == n_mm

    exp_sb = stat_pool.tile([R, BLOCK], f32)
    nc.scalar.activation(exp_sb[:], d_psum[:],
                         mybir.ActivationFunctionType.Exp)

    rcat = stat_pool.tile([R, 4], f32)
    srcs = [d_psum, d_psum, exp_sb, exp_sb]
    for f in range(4):
        scratch = stat_pool.tile([R, BLOCK], f32, tag="scr")
        nc.vector.tensor_mul(scratch[:], srcs[f][:], m_tiles[f][:])
        nc.vector.reduce_sum(rcat[:, f:f + 1], scratch[:],
                             axis=mybir.AxisListType.X)

    f_psum = psum_pool.tile([1, 4], f32, tag="final")
    nc.tensor.matmul(f_psum[:], ones_f32[0:R, :], rcat[:], start=True, stop=True)
    res = stat_pool.tile([1, 4], f32)
    nc.scalar.copy(res[:], f_psum[:])
    nc.sync.dma_start(out, res[:])


_CACHE = {}


def _compiled(S, nb, dt_name, prog_opts=None):
    key = (S, nb, dt_name, tuple(sorted((prog_opts or {}).items(),
                                        key=lambda kv: kv[0])))
    if key in _CACHE:
        return _CACHE[key]
    from contextlib import ExitStack

    import concourse.bacc as bacc
    import concourse.mybir as mybir
    import concourse.tile as tile

    dt_in = getattr(mybir.dt, dt_name)
    nc = bacc.Bacc("TRN2", target_bir_lowering=False, debug=False,
                   num_devices=N_CORES)
    with tile.TileContext(nc) as tc:
        with ExitStack() as ctx:
            trace_program(nc, tc, ctx, S, nb, dt_in, **(prog_opts or {}))
    nc.compile()
    _CACHE[key] = nc
    return nc


# -------------------------------------------------------------------- kernel

def kernel(x, y, _dt_name="float16", _run_opts=None, _prog_opts=None):
    x = np.asarray(x)
    y = np.asarray(y)
    assert x.shape == (B, C, 256, 256) and y.shape == (N,)

    colmap, fams, P, Ng, M = _build_walk(y)
    E = fams.shape[0]
    S = E // N_CORES
    nb = S // BLOCK
    assert nb * BLOCK * N_CORES == E and 4 * nb <= 128

    np_dt = {"float32": np.float32, "float16": np.float16}[_dt_name]
    x3 = x.reshape(B, C, N)
    xp = np.ascontiguousarray(x3[:, :, colmap], dtype=np_dt)  # [B, C, E+1]

    # masks in d-row layout: row 4k+b of core i covers edges
    # i*S + k*BLOCK + [0, BLOCK), identical for the 4 b rows
    fams_c = fams.reshape(N_CORES, nb, 1, BLOCK)
    m = np.empty((N_CORES, 4, 4 * nb, BLOCK), dtype=np.float16)
    for f in range(4):
        m[:, f] = np.broadcast_to(fams_c == f + 1,
                                  (N_CORES, nb, 4, BLOCK)
                                  ).reshape(N_CORES, 4 * nb, BLOCK)

    in_maps = [
        {"xp": np.ascontiguousarray(xp[:, :, i * S:(i + 1) * S + 1]),
         "msk": m[i]}
        for i in range(N_CORES)
    ]

    nc = _compiled(S, nb, _dt_name, _prog_opts)
    from concourse.bass_utils import run_bass_kernel_spmd

    res = run_bass_kernel_spmd(nc, in_maps, list(range(N_CORES)),
                               **(_run_opts or {}))
    partials = np.stack([r["out"][0] for r in res.results])  # [N_CORES, 4]
    s1, s2, s3, s4 = partials.sum(axis=0, dtype=np.float64)

    n = float(B * M)
    loss = (-s1 / (B * P) - s2 / (B * Ng)
            + math.log(s3) - math.log(n) + math.log(s4) - math.log(n))
    assert np.isfinite(loss)
    out = np.float32(loss)
    if _run_opts:
        return out, res
    return out


# revision 11
# speedup vs baseline: 1.2320x; 1.1324x over previous
"""Trainium2 Bass kernel for nn_ContrastiveLoss (B=4, C=256, H=W=256).

Strategy
--------
The reference computes four families of per-position channel dot products
over columns of x viewed as [B, C, N] (N = H*W), then scalar reductions:

  fam1 (pos_sim): dot(x[:,:,pos[t]],  x[:,:,pos[t+P]])   t in [0,P)
  fam2 (neg_sim): dot(x[:,:,neg[t]],  x[:,:,neg[t+Ng]])  t in [0,Ng)
  fam3 (pn1):     dot(x[:,:,pos[t]],  x[:,:,neg[t]])     t in [0,M)
  fam4 (pn2):     dot(x[:,:,pos[t]],  x[:,:,neg[t]])     t in [M,2M)

Each column of x participates in at most two dot products, so the union of
the four pairings is a degree-<=2 graph = disjoint paths and even cycles.
The host walks those paths/cycles and emits columns in walk order; in the
permuted tensor xp every dot product is between ADJACENT columns.  The
device streams xp once (~17 MB/core fp16, the HBM roofline ~47us), computes
shifted products xp[:,:,i]*xp[:,:,i+1], reduces over C via a ones-staircase
matmul on the tensor engine (PSUM-accumulated over the two 128-channel
chunks), and applies per-family 0/1 masks to form the four partial
reductions.  Cycles are closed by re-emitting their first column; junk
edges between components and in padding have all-zero masks.

Engine assignment (measured on HW, not theory):
 * ALL products on the DVE in fp16 2x mode (~1.2us per [128,4096] tile).
   Offloading any multiplies to GpSimd is a large net loss: a concurrent
   GpSimd tensor op knocks the DVE from 1224ns to 4616ns per tile via SBUF
   port contention.
 * Staircase matmuls run ~380-400ns (not the 213ns ideal): the PE p-state
   only ramps during long uninterrupted busy streaks.  LDWEIGHTS (~110ns)
   hides under the preceding matmul.  16 warmup matmuls on junk data keep
   the PE busy while the first DMA tiles land.  Consecutive matmuls
   alternate between two PSUM accumulation chains (psum_chains=2) to avoid
   same-bank accumulate turnaround; the chains are summed at the end.
 * Chunk-combining products with a DVE add (to halve matmul count) was
   tried and is a net loss: the add serializes behind both muls and starves
   the PE.
 * Per-core HW time ~69-77us (run-to-run spread ~5us), vs 93.5us for the
   GpSimd-offload baseline.

Sharding: the edge list is split into 8 equal contiguous chunks of the
column walk (one per NeuronCore, overlapping by one column).  Each core
returns 4 partial scalars (sum d*m1, sum d*m2, sum exp(d)*m3, sum
exp(d)*m4); the host combines them into the final loss.  exp() needs no
max-subtraction: |d| < ~0.5 for this data regime, so sum(exp(d)) is stable
in fp32 (guarded by an assert on the host).
"""

import math
import sys

import numpy as np

if "/opt/trn_rl_repo" not in sys.path:  # harness runs from a fresh dir
    sys.path.insert(0, "/opt/trn_rl_repo")

B, C, N = 4, 256, 65536
N_CORES = 8
BLOCK = 512          # edges per PSUM block (= max fp32 matmul free dim)
CHUNKS = C // 128    # channel chunks of 128 partitions


# ---------------------------------------------------------------- host prep

def _build_walk(y):
    """Column permutation + per-edge family labels (0 = junk/padding)."""
    y = np.asarray(y).reshape(-1)
    pos_idx = np.nonzero(y == 1)[0]
    neg_idx = np.nonzero(y == 0)[0]
    P = pos_idx.shape[0] // 2
    Ng = neg_idx.shape[0] // 2
    M = min(P, Ng)

    nP, nN = 2 * P, 2 * Ng
    V = nP + nN
    t_pos = np.arange(nP)
    t_neg = np.arange(nN)
    nbrA = np.empty(V, dtype=np.int64)
    nbrA[:nP] = np.where(t_pos < P, t_pos + P, t_pos - P)
    nbrA[nP:] = nP + np.where(t_neg < Ng, t_neg + Ng, t_neg - Ng)
    famA = np.empty(V, dtype=np.int8)
    famA[:nP] = 1
    famA[nP:] = 2
    nbrB = np.full(V, -1, dtype=np.int64)
    nbrB[:2 * M] = nP + t_pos[:2 * M]
    nbrB[nP:nP + 2 * M] = t_neg[:2 * M]
    famB = np.zeros(V, dtype=np.int8)
    famB[:M] = 3
    famB[M:2 * M] = 4
    famB[nP:nP + M] = 3
    famB[nP + M:nP + 2 * M] = 4

    visited = np.zeros(V, dtype=bool)
    order = np.empty(V + V // 4 + 16, dtype=np.int64)
    fams_l = np.empty(order.shape[0], dtype=np.int8)
    no = 0
    ne = 0

    def walk_from(v0, is_cycle):
        nonlocal no, ne
        if no > 0:
            fams_l[ne] = 0  # junk edge joining the previous component
            ne += 1
        v = v0
        use_A = True  # endpoints/cycle starts leave via their A edge
        order[no] = v
        no += 1
        visited[v] = True
        while True:
            if use_A:
                nxt, fam = nbrA[v], famA[v]
            else:
                nxt = nbrB[v]
                if nxt < 0:
                    return
                fam = famB[v]
            if visited[nxt]:
                if is_cycle and nxt == v0 and not use_A:
                    fams_l[ne] = fam
                    ne += 1
                    order[no] = v0  # close the cycle
                    no += 1
                return
            fams_l[ne] = fam
            ne += 1
            order[no] = nxt
            no += 1
            visited[nxt] = True
            v = nxt
            use_A = not use_A

    for v0 in np.nonzero(nbrB < 0)[0]:
        if not visited[v0]:
            walk_from(int(v0), is_cycle=False)
    for v0 in range(V):
        if not visited[v0]:
            walk_from(int(v0), is_cycle=True)

    n_real = int((fams_l[:ne] > 0).sum())
    assert n_real == P + Ng + 2 * M, (n_real, P + Ng + 2 * M)

    per = N_CORES * BLOCK
    E_pad = ((ne + per - 1) // per) * per
    V_pad = E_pad + 1
    fams = np.zeros(E_pad, dtype=np.int8)
    fams[:ne] = fams_l[:ne]
    vert = np.zeros(V_pad, dtype=np.int64)
    vert[:no] = order[:no]
    colmap = np.where(vert < nP, pos_idx[np.minimum(vert, nP - 1)],
                      neg_idx[np.maximum(vert - nP, 0)])
    return colmap, fams, P, Ng, M


# ------------------------------------------------------------- device program

def trace_program(nc, tc, ctx, S, nb, dt_in, **prog_opts):
    """Emit the per-core program. S = edges/core, nb = S//BLOCK.

    DRAM tensors (per core): xp [B, C, S+1] dt_in, msk [4, 4*nb, BLOCK] f32,
    out [1, 4] f32 = (sum d*m1, sum d*m2, sum exp(d)*m3, sum exp(d)*m4).
    """
    import concourse.mybir as mybir

    f32 = mybir.dt.float32
    R = 4 * nb
    xp = nc.dram_tensor("xp", [B, C, S + 1], dt_in, kind="ExternalInput").ap()
    msk = nc.dram_tensor("msk", [4, R, BLOCK], f32, kind="ExternalInput").ap()
    out = nc.dram_tensor("out", [1, 4], f32, kind="ExternalOutput").ap()
    trace_program_aps(nc, tc, ctx, S, nb, dt_in, xp, msk, out, **prog_opts)


def trace_program_aps(nc, tc, ctx, S, nb, dt_in, xp, msk, out,
                      gpsimd_on=(), xp_bufs=12, prod_bufs=10,
                      kb_blocks=8, psum_chains=2, warmup=16):
    import concourse.mybir as mybir

    f32 = mybir.dt.float32
    R = 4 * nb  # d rows: block k, batch b -> row 4k+b

    const_pool = ctx.enter_context(tc.tile_pool(name="const", bufs=1))
    mask_pool = ctx.enter_context(tc.tile_pool(name="masks", bufs=1))
    xp_pool = ctx.enter_context(tc.tile_pool(name="xp", bufs=xp_bufs))
    prod_pool = ctx.enter_context(tc.tile_pool(name="prod", bufs=prod_bufs))
    stat_pool = ctx.enter_context(tc.tile_pool(name="stat", bufs=1))
    psum_pool = ctx.enter_context(tc.tile_pool(name="psum", bufs=1, space="PSUM"))

    # Staircase selector: zo[:, 63] = 1, else 0.  lhsT = zo[:, 63-r : 127-r]
    # is a [128, R] one-hot-column matrix that routes a partition-dim
    # column-sum into PSUM row r (other rows accumulate exact zeros) --
    # matmul PSUM outputs must start at partition 0/32/64, so rows can't be
    # addressed via the output AP.
    dt_prod = dt_in  # DVE converts on write for free; PE runs 16-bit at rate
    junk = const_pool.tile([128, BLOCK], dt_prod)
    nc.vector.memset(junk[:], 0.0)
    zo = const_pool.tile([128, 63 + R], dt_prod)
    nc.vector.memset(zo[:], 0.0)
    nc.vector.memset(zo[:, 63:64], 1.0)
    ones_f32 = const_pool.tile([128, 1], f32)
    nc.vector.memset(ones_f32[:], 1.0)

    if warmup:
        jp = psum_pool.tile([32, BLOCK], mybir.dt.float32, tag="junkp",
                            name="junk_psum")
        for _ in range(warmup):
            # keeps the PE p-state ramped while real products are not ready
            nc.tensor.matmul(jp[:, :], junk[:, 0:32], junk[:, 0:BLOCK],
                             start=True, stop=True, skip_group_check=True)
    m_tiles = []
    for f in range(4):
        mt = mask_pool.tile([R, BLOCK], f32, tag=f"m{f}")
        nc.sync.dma_start(mt[:], msk[f])
        m_tiles.append(mt)

    d_psums = [psum_pool.tile([R, BLOCK], f32, tag=f"d{i}", name=f"d_psum{i}")
               for i in range(psum_chains)]

    # KB 512-edge blocks per DMA/mul tile: fewer, larger DMAs and DVE ops
    KB = kb_blocks or (4 if nb % 4 == 0 else (2 if nb % 2 == 0 else 1))
    assert nb % KB == 0
    W = KB * BLOCK
    n_mm = nb * B * CHUNKS
    mm_per_chain = n_mm // psum_chains
    assert n_mm % psum_chains == 0
    chain_cnt = [0] * psum_chains
    i_mm = 0
    mul_i = 0
    for kb in range(nb // KB):
        for b in range(B):
            prods = []
            for c in range(CHUNKS):
                t = xp_pool.tile([128, W + 1], dt_in)
                nc.sync.dma_start(
                    t[:], xp[b, 128 * c:128 * (c + 1), W * kb:W * (kb + 1) + 1])
                p = prod_pool.tile([128, W], dt_prod)
                # DVE is the mul bottleneck; GpSimd runs these ~1.7x
                # slower, so balance ~3/8 of them onto it
                eng = nc.gpsimd if (gpsimd_on and
                                    mul_i % 8 in gpsimd_on) else nc.vector
                eng.tensor_mul(p[:], t[:, 0:W], t[:, 1:W + 1])
                mul_i += 1
                prods.append(p)
            for j in range(KB):
                row = 4 * (kb * KB + j) + b
                for c in range(CHUNKS):
                    ch = i_mm % psum_chains
                    nc.tensor.matmul(
                        d_psums[ch][:, :], zo[:, 63 - row:63 - row + R],
                        prods[c][:, BLOCK * j:BLOCK * (j + 1)],
                        start=(chain_cnt[ch] == 0),
                        stop=(chain_cnt[ch] == mm_per_chain - 1))
                    chain_cnt[ch] += 1
                    i_mm += 1

    if psum_chains == 1:
        d_fin = d_psums[0]
    else:
        d_fin = stat_pool.tile([R, BLOCK], f32, tag="dfin")
        nc.scalar.copy(d_fin[:], d_psums[0][:])
        for i in range(1, psum_chains):
            nc.vector.tensor_add(d_fin[:], d_fin[:], d_psums[i][:])
    exp_sb = stat_pool.tile([R, BLOCK], f32)
    nc.scalar.activation(exp_sb[:], d_fin[:],
                         mybir.ActivationFunctionType.Exp)

    rcat = stat_pool.tile([R, 4], f32)
    srcs = [d_fin, d_fin, exp_sb, exp_sb]
    for f in range(4):
        scratch = stat_pool.tile([R, BLOCK], f32, tag="scr")
        nc.vector.tensor_mul(scratch[:], srcs[f][:], m_tiles[f][:])
        nc.vector.reduce_sum(rcat[:, f:f + 1], scratch[:],
                             axis=mybir.AxisListType.X)

    f_psum = psum_pool.tile([1, 4], f32, tag="final")
    nc.tensor.matmul(f_psum[:], ones_f32[0:R, :], rcat[:], start=True, stop=True)
    res = stat_pool.tile([1, 4], f32)
    nc.scalar.copy(res[:], f_psum[:])
    nc.sync.dma_start(out, res[:])


_CACHE = {}


def _compiled(S, nb, dt_name, prog_opts=None):
    key = (S, nb, dt_name, repr(sorted((prog_opts or {}).items(),
                                       key=lambda kv: kv[0])))
    if key in _CACHE:
        return _CACHE[key]
    from contextlib import ExitStack

    import concourse.bacc as bacc
    import concourse.mybir as mybir
    import concourse.tile as tile

    dt_in = getattr(mybir.dt, dt_name)
    nc = bacc.Bacc("TRN2", target_bir_lowering=False, debug=False,
                   num_devices=N_CORES)
    with tile.TileContext(nc) as tc:
        with ExitStack() as ctx:
            trace_program(nc, tc, ctx, S, nb, dt_in, **(prog_opts or {}))
    nc.compile()
    _CACHE[key] = nc
    return nc


# -------------------------------------------------------------------- kernel

def kernel(x, y, _dt_name="float16", _run_opts=None, _prog_opts=None):
    x = np.asarray(x)
    y = np.asarray(y)
    assert x.shape == (B, C, 256, 256) and y.shape == (N,)

    colmap, fams, P, Ng, M = _build_walk(y)
    E = fams.shape[0]
    S = E // N_CORES
    nb = S // BLOCK
    assert nb * BLOCK * N_CORES == E and 4 * nb <= 128

    np_dt = {"float32": np.float32, "float16": np.float16}[_dt_name]
    x3 = x.reshape(B, C, N)
    xp = np.ascontiguousarray(x3[:, :, colmap], dtype=np_dt)  # [B, C, E+1]

    # masks in d-row layout: row 4k+b of core i covers edges
    # i*S + k*BLOCK + [0, BLOCK), identical for the 4 b rows
    fams_c = fams.reshape(N_CORES, nb, 1, BLOCK)
    m = np.empty((N_CORES, 4, 4 * nb, BLOCK), dtype=np.float32)
    for f in range(4):
        m[:, f] = np.broadcast_to(fams_c == f + 1,
                                  (N_CORES, nb, 4, BLOCK)
                                  ).reshape(N_CORES, 4 * nb, BLOCK)

    in_maps = [
        {"xp": np.ascontiguousarray(xp[:, :, i * S:(i + 1) * S + 1]),
         "msk": m[i]}
        for i in range(N_CORES)
    ]

    nc = _compiled(S, nb, _dt_name, _prog_opts)
    from concourse.bass_utils import run_bass_kernel_spmd

    res = run_bass_kernel_spmd(nc, in_maps, list(range(N_CORES)),
                               **(_run_opts or {}))
    partials = np.stack([r["out"][0] for r in res.results])  # [N_CORES, 4]
    s1, s2, s3, s4 = partials.sum(axis=0, dtype=np.float64)

    n = float(B * M)
    loss = (-s1 / (B * P) - s2 / (B * Ng)
            + math.log(s3) - math.log(n) + math.log(s4) - math.log(n))
    assert np.isfinite(loss)
    out = np.float32(loss)
    if _run_opts:
        return out, res
    return out



# revision 12
# speedup vs baseline: 1.2578x; 1.0210x over previous
"""Trainium2 Bass kernel for nn_ContrastiveLoss (B=4, C=256, H=W=256).

Strategy
--------
The reference computes four families of per-position channel dot products
over columns of x viewed as [B, C, N] (N = H*W), then scalar reductions:

  fam1 (pos_sim): dot(x[:,:,pos[t]],  x[:,:,pos[t+P]])   t in [0,P)
  fam2 (neg_sim): dot(x[:,:,neg[t]],  x[:,:,neg[t+Ng]])  t in [0,Ng)
  fam3 (pn1):     dot(x[:,:,pos[t]],  x[:,:,neg[t]])     t in [0,M)
  fam4 (pn2):     dot(x[:,:,pos[t]],  x[:,:,neg[t]])     t in [M,2M)

Each column of x participates in at most two dot products, so the union of
the four pairings is a degree-<=2 graph = disjoint paths and even cycles.
The host walks those paths/cycles and emits columns in walk order; in the
permuted tensor xp every dot product is between ADJACENT columns.  The
device streams xp once (~17 MB/core fp16, the HBM roofline ~47us), computes
shifted products xp[:,:,i]*xp[:,:,i+1], reduces over C via a ones-staircase
matmul on the tensor engine (PSUM-accumulated over the two 128-channel
chunks), and applies per-family 0/1 masks to form the four partial
reductions.  Cycles are closed by re-emitting their first column; junk
edges between components and in padding have all-zero masks.

Engine assignment (measured on HW, not theory):
 * ALL products on the DVE in fp16 2x mode (~1.2us per [128,4096] tile).
   Offloading any multiplies to GpSimd is a large net loss: a concurrent
   GpSimd tensor op knocks the DVE from 1224ns to 4616ns per tile via SBUF
   port contention.
 * Staircase matmuls run ~380-400ns (not the 213ns ideal): the PE p-state
   only ramps during long uninterrupted busy streaks.  LDWEIGHTS (~110ns)
   hides under the preceding matmul.  16 warmup matmuls on junk data keep
   the PE busy while the first DMA tiles land.  Consecutive matmuls
   alternate between two PSUM accumulation chains (psum_chains=2) to avoid
   same-bank accumulate turnaround; the chains are summed at the end.
 * Chunk-combining products with a DVE add (to halve matmul count) was
   tried and is a net loss: the add serializes behind both muls and starves
   the PE.
 * Per-core HW time ~69-77us (run-to-run spread ~5us), vs 93.5us for the
   GpSimd-offload baseline.

Sharding: the edge list is split into 8 equal contiguous chunks of the
column walk (one per NeuronCore, overlapping by one column).  Each core
returns 4 partial scalars (sum d*m1, sum d*m2, sum exp(d)*m3, sum
exp(d)*m4); the host combines them into the final loss.  exp() needs no
max-subtraction: |d| < ~0.5 for this data regime, so sum(exp(d)) is stable
in fp32 (guarded by an assert on the host).
"""

import math
import sys

import numpy as np

if "/opt/trn_rl_repo" not in sys.path:  # harness runs from a fresh dir
    sys.path.insert(0, "/opt/trn_rl_repo")

B, C, N = 4, 256, 65536
N_CORES = 8
BLOCK = 512          # edges per PSUM block (= max fp32 matmul free dim)
CHUNKS = C // 128    # channel chunks of 128 partitions


# ---------------------------------------------------------------- host prep

def _build_walk(y):
    """Column permutation + per-edge family labels (0 = junk/padding)."""
    y = np.asarray(y).reshape(-1)
    pos_idx = np.nonzero(y == 1)[0]
    neg_idx = np.nonzero(y == 0)[0]
    P = pos_idx.shape[0] // 2
    Ng = neg_idx.shape[0] // 2
    M = min(P, Ng)

    nP, nN = 2 * P, 2 * Ng
    V = nP + nN
    t_pos = np.arange(nP)
    t_neg = np.arange(nN)
    nbrA = np.empty(V, dtype=np.int64)
    nbrA[:nP] = np.where(t_pos < P, t_pos + P, t_pos - P)
    nbrA[nP:] = nP + np.where(t_neg < Ng, t_neg + Ng, t_neg - Ng)
    famA = np.empty(V, dtype=np.int8)
    famA[:nP] = 1
    famA[nP:] = 2
    nbrB = np.full(V, -1, dtype=np.int64)
    nbrB[:2 * M] = nP + t_pos[:2 * M]
    nbrB[nP:nP + 2 * M] = t_neg[:2 * M]
    famB = np.zeros(V, dtype=np.int8)
    famB[:M] = 3
    famB[M:2 * M] = 4
    famB[nP:nP + M] = 3
    famB[nP + M:nP + 2 * M] = 4

    visited = np.zeros(V, dtype=bool)
    order = np.empty(V + V // 4 + 16, dtype=np.int64)
    fams_l = np.empty(order.shape[0], dtype=np.int8)
    no = 0
    ne = 0

    def walk_from(v0, is_cycle):
        nonlocal no, ne
        if no > 0:
            fams_l[ne] = 0  # junk edge joining the previous component
            ne += 1
        v = v0
        use_A = True  # endpoints/cycle starts leave via their A edge
        order[no] = v
        no += 1
        visited[v] = True
        while True:
            if use_A:
                nxt, fam = nbrA[v], famA[v]
            else:
                nxt = nbrB[v]
                if nxt < 0:
                    return
                fam = famB[v]
            if visited[nxt]:
                if is_cycle and nxt == v0 and not use_A:
                    fams_l[ne] = fam
                    ne += 1
                    order[no] = v0  # close the cycle
                    no += 1
                return
            fams_l[ne] = fam
            ne += 1
            order[no] = nxt
            no += 1
            visited[nxt] = True
            v = nxt
            use_A = not use_A

    for v0 in np.nonzero(nbrB < 0)[0]:
        if not visited[v0]:
            walk_from(int(v0), is_cycle=False)
    for v0 in range(V):
        if not visited[v0]:
            walk_from(int(v0), is_cycle=True)

    n_real = int((fams_l[:ne] > 0).sum())
    assert n_real == P + Ng + 2 * M, (n_real, P + Ng + 2 * M)

    per = N_CORES * BLOCK
    E_pad = ((ne + per - 1) // per) * per
    V_pad = E_pad + 1
    fams = np.zeros(E_pad, dtype=np.int8)
    fams[:ne] = fams_l[:ne]
    vert = np.zeros(V_pad, dtype=np.int64)
    vert[:no] = order[:no]
    colmap = np.where(vert < nP, pos_idx[np.minimum(vert, nP - 1)],
                      neg_idx[np.maximum(vert - nP, 0)])
    return colmap, fams, P, Ng, M


# ------------------------------------------------------------- device program

def trace_program(nc, tc, ctx, S, nb, dt_in, **prog_opts):
    """Emit the per-core program. S = edges/core, nb = S//BLOCK.

    DRAM tensors (per core): xp [B, C, S+1] dt_in, msk [4, 4*nb, BLOCK] f32,
    out [1, 4] f32 = (sum d*m1, sum d*m2, sum exp(d)*m3, sum exp(d)*m4).
    """
    import concourse.mybir as mybir

    f32 = mybir.dt.float32
    R = 4 * nb
    xp = nc.dram_tensor("xp", [B, C, S + 1], dt_in, kind="ExternalInput").ap()
    msk = nc.dram_tensor("msk", [4, R, BLOCK], f32, kind="ExternalInput").ap()
    out = nc.dram_tensor("out", [1, 4], f32, kind="ExternalOutput").ap()
    trace_program_aps(nc, tc, ctx, S, nb, dt_in, xp, msk, out, **prog_opts)


def trace_program_aps(nc, tc, ctx, S, nb, dt_in, xp, msk, out,
                      gpsimd_on=(), xp_bufs=12, prod_bufs=10,
                      kb_blocks=8, psum_chains=2, warmup=16):
    import concourse.mybir as mybir

    f32 = mybir.dt.float32
    R = 4 * nb  # d rows: block k, batch b -> row 4k+b

    const_pool = ctx.enter_context(tc.tile_pool(name="const", bufs=1))
    mask_pool = ctx.enter_context(tc.tile_pool(name="masks", bufs=1))
    xp_pool = ctx.enter_context(tc.tile_pool(name="xp", bufs=xp_bufs))
    prod_pool = ctx.enter_context(tc.tile_pool(name="prod", bufs=prod_bufs))
    stat_pool = ctx.enter_context(tc.tile_pool(name="stat", bufs=1))
    psum_pool = ctx.enter_context(tc.tile_pool(name="psum", bufs=1, space="PSUM"))

    # Staircase selector: zo[:, 63] = 1, else 0.  lhsT = zo[:, 63-r : 127-r]
    # is a [128, R] one-hot-column matrix that routes a partition-dim
    # column-sum into PSUM row r (other rows accumulate exact zeros) --
    # matmul PSUM outputs must start at partition 0/32/64, so rows can't be
    # addressed via the output AP.
    dt_prod = dt_in  # DVE converts on write for free; PE runs 16-bit at rate
    junk = const_pool.tile([128, BLOCK], dt_prod)
    nc.vector.memset(junk[:], 0.0)
    zo = const_pool.tile([128, 63 + R], dt_prod)
    nc.vector.memset(zo[:], 0.0)
    nc.vector.memset(zo[:, 63:64], 1.0)
    ones_f32 = const_pool.tile([128, 1], f32)
    nc.vector.memset(ones_f32[:], 1.0)

    if warmup:
        jp = psum_pool.tile([32, BLOCK], mybir.dt.float32, tag="junkp",
                            name="junk_psum")
        for _ in range(warmup):
            # keeps the PE p-state ramped while real products are not ready
            nc.tensor.matmul(jp[:, :], junk[:, 0:32], junk[:, 0:BLOCK],
                             start=True, stop=True, skip_group_check=True)
    m_tiles = []
    for f in range(4):
        mt = mask_pool.tile([R, BLOCK], f32, tag=f"m{f}")
        nc.sync.dma_start(mt[:], msk[f])
        m_tiles.append(mt)

    d_psums = [psum_pool.tile([R, BLOCK], f32, tag=f"d{i}", name=f"d_psum{i}")
               for i in range(psum_chains)]

    # KB 512-edge blocks per DMA/mul tile: fewer, larger DMAs and DVE ops
    KB = kb_blocks or (4 if nb % 4 == 0 else (2 if nb % 2 == 0 else 1))
    assert nb % KB == 0
    W = KB * BLOCK
    n_mm = nb * B * CHUNKS
    mm_per_chain = n_mm // psum_chains
    assert n_mm % psum_chains == 0
    chain_cnt = [0] * psum_chains
    i_mm = 0
    mul_i = 0
    for kb in range(nb // KB):
        for b in range(B):
            prods = []
            for c in range(CHUNKS):
                t = xp_pool.tile([128, W + 1], dt_in)
                nc.sync.dma_start(
                    t[:], xp[b, 128 * c:128 * (c + 1), W * kb:W * (kb + 1) + 1])
                p = prod_pool.tile([128, W], dt_prod)
                # DVE is the mul bottleneck; GpSimd runs these ~1.7x
                # slower, so balance ~3/8 of them onto it
                eng = nc.gpsimd if (gpsimd_on and
                                    mul_i % 8 in gpsimd_on) else nc.vector
                eng.tensor_mul(p[:], t[:, 0:W], t[:, 1:W + 1])
                mul_i += 1
                prods.append(p)
            for j in range(KB):
                row = 4 * (kb * KB + j) + b
                for c in range(CHUNKS):
                    ch = i_mm % psum_chains
                    nc.tensor.matmul(
                        d_psums[ch][:, :], zo[:, 63 - row:63 - row + R],
                        prods[c][:, BLOCK * j:BLOCK * (j + 1)],
                        start=(chain_cnt[ch] == 0),
                        stop=(chain_cnt[ch] == mm_per_chain - 1))
                    chain_cnt[ch] += 1
                    i_mm += 1

    if psum_chains == 1:
        d_fin = d_psums[0]
    else:
        d_fin = stat_pool.tile([R, BLOCK], f32, tag="dfin")
        nc.scalar.copy(d_fin[:], d_psums[0][:])
        for i in range(1, psum_chains):
            nc.vector.tensor_add(d_fin[:], d_fin[:], d_psums[i][:])
    exp_sb = stat_pool.tile([R, BLOCK], f32)
    nc.scalar.activation(exp_sb[:], d_fin[:],
                         mybir.ActivationFunctionType.Exp)

    # mask 0 is host-combined: -m1/(B*P) - m2/(B*Ng), so col 0 is the
    # whole linear term of the loss; mask 1 is unused on device
    rcat = stat_pool.tile([R, 4], f32)
    nc.vector.memset(rcat[:, 1:2], 0.0)
    srcs = {0: d_fin, 2: exp_sb, 3: exp_sb}
    for f in (0, 2, 3):
        scratch = stat_pool.tile([R, BLOCK], f32, tag="scr")
        nc.vector.tensor_mul(scratch[:], srcs[f][:], m_tiles[f][:])
        nc.vector.reduce_sum(rcat[:, f:f + 1], scratch[:],
                             axis=mybir.AxisListType.X)

    f_psum = psum_pool.tile([1, 4], f32, tag="final")
    nc.tensor.matmul(f_psum[:], ones_f32[0:R, :], rcat[:], start=True, stop=True)
    res = stat_pool.tile([1, 4], f32)
    nc.scalar.copy(res[:], f_psum[:])
    nc.sync.dma_start(out, res[:])


_CACHE = {}


def _compiled(S, nb, dt_name, prog_opts=None):
    key = (S, nb, dt_name, repr(sorted((prog_opts or {}).items(),
                                       key=lambda kv: kv[0])))
    if key in _CACHE:
        return _CACHE[key]
    from contextlib import ExitStack

    import concourse.bacc as bacc
    import concourse.mybir as mybir
    import concourse.tile as tile

    dt_in = getattr(mybir.dt, dt_name)
    nc = bacc.Bacc("TRN2", target_bir_lowering=False, debug=False,
                   num_devices=N_CORES)
    with tile.TileContext(nc) as tc:
        with ExitStack() as ctx:
            trace_program(nc, tc, ctx, S, nb, dt_in, **(prog_opts or {}))
    nc.compile()
    _CACHE[key] = nc
    return nc


# -------------------------------------------------------------------- kernel

def kernel(x, y, _dt_name="float16", _run_opts=None, _prog_opts=None):
    x = np.asarray(x)
    y = np.asarray(y)
    assert x.shape == (B, C, 256, 256) and y.shape == (N,)

    colmap, fams, P, Ng, M = _build_walk(y)
    E = fams.shape[0]
    S = E // N_CORES
    nb = S // BLOCK
    assert nb * BLOCK * N_CORES == E and 4 * nb <= 128

    np_dt = {"float32": np.float32, "float16": np.float16}[_dt_name]
    x3 = x.reshape(B, C, N)
    xp = np.ascontiguousarray(x3[:, :, colmap], dtype=np_dt)  # [B, C, E+1]

    # masks in d-row layout: row 4k+b of core i covers edges
    # i*S + k*BLOCK + [0, BLOCK), identical for the 4 b rows
    fams_c = fams.reshape(N_CORES, nb, 1, BLOCK)
    m = np.empty((N_CORES, 4, 4 * nb, BLOCK), dtype=np.float32)
    for f in range(4):
        m[:, f] = np.broadcast_to(fams_c == f + 1,
                                  (N_CORES, nb, 4, BLOCK)
                                  ).reshape(N_CORES, 4 * nb, BLOCK)
    # fold the two linear-family reductions into one device pass: col 0 of
    # the device result becomes the full linear term of the loss
    m[:, 0] = -m[:, 0] / (B * P) - m[:, 1] / (B * Ng)

    in_maps = [
        {"xp": np.ascontiguousarray(xp[:, :, i * S:(i + 1) * S + 1]),
         "msk": m[i]}
        for i in range(N_CORES)
    ]

    nc = _compiled(S, nb, _dt_name, _prog_opts)
    from concourse.bass_utils import run_bass_kernel_spmd

    res = run_bass_kernel_spmd(nc, in_maps, list(range(N_CORES)),
                               **(_run_opts or {}))
    partials = np.stack([r["out"][0] for r in res.results])  # [N_CORES, 4]
    lin, _, s3, s4 = partials.sum(axis=0, dtype=np.float64)

    n = float(B * M)
    loss = (lin
            + math.log(s3) - math.log(n) + math.log(s4) - math.log(n))
    assert np.isfinite(loss)
    out = np.float32(loss)
    if _run_opts:
        return out, res
    return out



# revision 13
# speedup vs baseline: 1.3726x; 1.0912x over previous
"""Trainium2 Bass kernel for nn_ContrastiveLoss (B=4, C=256, H=W=256).

Strategy
--------
The reference computes four families of per-position channel dot products
over columns of x viewed as [B, C, N] (N = H*W), then scalar reductions:

  fam1 (pos_sim): dot(x[:,:,pos[t]],  x[:,:,pos[t+P]])   t in [0,P)
  fam2 (neg_sim): dot(x[:,:,neg[t]],  x[:,:,neg[t+Ng]])  t in [0,Ng)
  fam3 (pn1):     dot(x[:,:,pos[t]],  x[:,:,neg[t]])     t in [0,M)
  fam4 (pn2):     dot(x[:,:,pos[t]],  x[:,:,neg[t]])     t in [M,2M)

Each column of x participates in at most two dot products, so the union of
the four pairings is a degree-<=2 graph = disjoint paths and even cycles.
The host walks those paths/cycles and emits columns in walk order; in the
permuted tensor xp every dot product is between ADJACENT columns.  The
device streams xp once (~17 MB/core fp16, the HBM roofline ~47us), computes
shifted products xp[:,:,i]*xp[:,:,i+1], reduces over C via a ones-staircase
matmul on the tensor engine (PSUM-accumulated over the two 128-channel
chunks), and applies per-family 0/1 masks to form the four partial
reductions.  Cycles are closed by re-emitting their first column; junk
edges between components and in padding have all-zero masks.

Engine assignment (measured on HW, not theory):
 * ALL products on the DVE in fp16 2x mode (~1.2us per [128,4096] tile).
   Offloading any multiplies to GpSimd is a large net loss: a concurrent
   GpSimd tensor op knocks the DVE from 1224ns to 4616ns per tile via SBUF
   port contention.
 * Staircase matmuls run ~380-400ns (not the 213ns ideal): the PE p-state
   only ramps during long uninterrupted busy streaks.  LDWEIGHTS (~110ns)
   hides under the preceding matmul.  16 warmup matmuls on junk data keep
   the PE busy while the first DMA tiles land.  Consecutive matmuls
   alternate between two PSUM accumulation chains (psum_chains=2) to avoid
   same-bank accumulate turnaround; the chains are summed at the end.
 * Chunk-combining products with a DVE add (to halve matmul count) was
   tried and is a net loss: the add serializes behind both muls and starves
   the PE.
 * Per-core HW time ~69-77us (run-to-run spread ~5us), vs 93.5us for the
   GpSimd-offload baseline.

Sharding: the edge list is split into 8 equal contiguous chunks of the
column walk (one per NeuronCore, overlapping by one column).  Each core
returns 4 partial scalars (sum d*m1, sum d*m2, sum exp(d)*m3, sum
exp(d)*m4); the host combines them into the final loss.  exp() needs no
max-subtraction: |d| < ~0.5 for this data regime, so sum(exp(d)) is stable
in fp32 (guarded by an assert on the host).
"""

import math
import sys

import numpy as np

if "/opt/trn_rl_repo" not in sys.path:  # harness runs from a fresh dir
    sys.path.insert(0, "/opt/trn_rl_repo")

B, C, N = 4, 256, 65536
N_CORES = 8
BLOCK = 512          # edges per PSUM block (= max fp32 matmul free dim)
CHUNKS = C // 128    # channel chunks of 128 partitions


# ---------------------------------------------------------------- host prep

def _build_walk(y):
    """Column permutation + per-edge family labels (0 = junk/padding)."""
    y = np.asarray(y).reshape(-1)
    pos_idx = np.nonzero(y == 1)[0]
    neg_idx = np.nonzero(y == 0)[0]
    P = pos_idx.shape[0] // 2
    Ng = neg_idx.shape[0] // 2
    M = min(P, Ng)

    nP, nN = 2 * P, 2 * Ng
    V = nP + nN
    t_pos = np.arange(nP)
    t_neg = np.arange(nN)
    nbrA = np.empty(V, dtype=np.int64)
    nbrA[:nP] = np.where(t_pos < P, t_pos + P, t_pos - P)
    nbrA[nP:] = nP + np.where(t_neg < Ng, t_neg + Ng, t_neg - Ng)
    famA = np.empty(V, dtype=np.int8)
    famA[:nP] = 1
    famA[nP:] = 2
    nbrB = np.full(V, -1, dtype=np.int64)
    nbrB[:2 * M] = nP + t_pos[:2 * M]
    nbrB[nP:nP + 2 * M] = t_neg[:2 * M]
    famB = np.zeros(V, dtype=np.int8)
    famB[:M] = 3
    famB[M:2 * M] = 4
    famB[nP:nP + M] = 3
    famB[nP + M:nP + 2 * M] = 4

    visited = np.zeros(V, dtype=bool)
    order = np.empty(V + V // 4 + 16, dtype=np.int64)
    fams_l = np.empty(order.shape[0], dtype=np.int8)
    no = 0
    ne = 0

    def walk_from(v0, is_cycle):
        nonlocal no, ne
        if no > 0:
            fams_l[ne] = 0  # junk edge joining the previous component
            ne += 1
        v = v0
        use_A = True  # endpoints/cycle starts leave via their A edge
        order[no] = v
        no += 1
        visited[v] = True
        while True:
            if use_A:
                nxt, fam = nbrA[v], famA[v]
            else:
                nxt = nbrB[v]
                if nxt < 0:
                    return
                fam = famB[v]
            if visited[nxt]:
                if is_cycle and nxt == v0 and not use_A:
                    fams_l[ne] = fam
                    ne += 1
                    order[no] = v0  # close the cycle
                    no += 1
                return
            fams_l[ne] = fam
            ne += 1
            order[no] = nxt
            no += 1
            visited[nxt] = True
            v = nxt
            use_A = not use_A

    for v0 in np.nonzero(nbrB < 0)[0]:
        if not visited[v0]:
            walk_from(int(v0), is_cycle=False)
    for v0 in range(V):
        if not visited[v0]:
            walk_from(int(v0), is_cycle=True)

    n_real = int((fams_l[:ne] > 0).sum())
    assert n_real == P + Ng + 2 * M, (n_real, P + Ng + 2 * M)

    per = N_CORES * BLOCK
    E_pad = ((ne + per - 1) // per) * per
    V_pad = E_pad + 1
    fams = np.zeros(E_pad, dtype=np.int8)
    fams[:ne] = fams_l[:ne]
    vert = np.zeros(V_pad, dtype=np.int64)
    vert[:no] = order[:no]
    colmap = np.where(vert < nP, pos_idx[np.minimum(vert, nP - 1)],
                      neg_idx[np.maximum(vert - nP, 0)])
    return colmap, fams, P, Ng, M


# ------------------------------------------------------------- device program

def trace_program(nc, tc, ctx, S, nb, dt_in, **prog_opts):
    """Emit the per-core program. S = edges/core, nb = S//BLOCK.

    DRAM tensors (per core): xp [B, C, S+1] dt_in, msk [4, 4*nb, BLOCK] f32,
    out [1, 4] f32 = (sum d*m1, sum d*m2, sum exp(d)*m3, sum exp(d)*m4).
    """
    import concourse.mybir as mybir

    f32 = mybir.dt.float32
    R = 4 * nb
    xp = nc.dram_tensor("xp", [B, C, S + 1], dt_in, kind="ExternalInput").ap()
    msk = nc.dram_tensor("msk", [4, R, BLOCK], f32, kind="ExternalInput").ap()
    out = nc.dram_tensor("out", [1, 4], f32, kind="ExternalOutput").ap()
    trace_program_aps(nc, tc, ctx, S, nb, dt_in, xp, msk, out, **prog_opts)


def trace_program_aps(nc, tc, ctx, S, nb, dt_in, xp, msk, out,
                      gpsimd_on=(), xp_bufs=12, prod_bufs=10,
                      kb_blocks=8, psum_chains=2, warmup=16):
    import concourse.mybir as mybir

    f32 = mybir.dt.float32
    R = 4 * nb  # d rows: block k, batch b -> row 4k+b

    const_pool = ctx.enter_context(tc.tile_pool(name="const", bufs=1))
    mask_pool = ctx.enter_context(tc.tile_pool(name="masks", bufs=1))
    xp_pool = ctx.enter_context(tc.tile_pool(name="xp", bufs=xp_bufs))
    prod_pool = ctx.enter_context(tc.tile_pool(name="prod", bufs=prod_bufs))
    stat_pool = ctx.enter_context(tc.tile_pool(name="stat", bufs=1))
    psum_pool = ctx.enter_context(tc.tile_pool(name="psum", bufs=1, space="PSUM"))

    # Staircase selector: zo[:, 63] = 1, else 0.  lhsT = zo[:, 63-r : 127-r]
    # is a [128, R] one-hot-column matrix that routes a partition-dim
    # column-sum into PSUM row r (other rows accumulate exact zeros) --
    # matmul PSUM outputs must start at partition 0/32/64, so rows can't be
    # addressed via the output AP.
    dt_prod = dt_in  # DVE converts on write for free; PE runs 16-bit at rate
    junk = const_pool.tile([128, BLOCK], dt_prod)
    nc.vector.memset(junk[:], 0.0)
    zo = const_pool.tile([128, 63 + R], dt_prod)
    nc.vector.memset(zo[:], 0.0)
    nc.vector.memset(zo[:, 63:64], 1.0)
    ones_f32 = const_pool.tile([128, 1], f32)
    nc.vector.memset(ones_f32[:], 1.0)

    if warmup:
        jp = psum_pool.tile([32, BLOCK], mybir.dt.float32, tag="junkp",
                            name="junk_psum")
        for _ in range(warmup):
            # keeps the PE p-state ramped while real products are not ready
            nc.tensor.matmul(jp[:, :], junk[:, 0:32], junk[:, 0:BLOCK],
                             start=True, stop=True, skip_group_check=True)
    m_tiles = []
    for f in range(4):
        mt = mask_pool.tile([R, BLOCK], f32, tag=f"m{f}")
        nc.sync.dma_start(mt[:], msk[f])
        m_tiles.append(mt)

    d_psums = [psum_pool.tile([R, BLOCK], f32, tag=f"d{i}", name=f"d_psum{i}")
               for i in range(psum_chains)]

    # KB 512-edge blocks per DMA/mul tile: fewer, larger DMAs and DVE ops
    KB = kb_blocks or (4 if nb % 4 == 0 else (2 if nb % 2 == 0 else 1))
    assert nb % KB == 0
    W = KB * BLOCK
    n_mm = nb * B * CHUNKS
    mm_per_chain = n_mm // psum_chains
    assert n_mm % psum_chains == 0
    chain_cnt = [0] * psum_chains
    i_mm = 0
    mul_i = 0
    for kb in range(nb // KB):
        for b in range(B):
            prods = []
            for c in range(CHUNKS):
                t = xp_pool.tile([128, W + 1], dt_in)
                nc.sync.dma_start(
                    t[:], xp[b, 128 * c:128 * (c + 1), W * kb:W * (kb + 1) + 1])
                p = prod_pool.tile([128, W], dt_prod)
                # DVE is the mul bottleneck; GpSimd runs these ~1.7x
                # slower, so balance ~3/8 of them onto it
                eng = nc.gpsimd if (gpsimd_on and
                                    mul_i % 8 in gpsimd_on) else nc.vector
                eng.tensor_mul(p[:], t[:, 0:W], t[:, 1:W + 1])
                mul_i += 1
                prods.append(p)
            for j in range(KB):
                row = 4 * (kb * KB + j) + b
                for c in range(CHUNKS):
                    ch = i_mm % psum_chains
                    nc.tensor.matmul(
                        d_psums[ch][:, :], zo[:, 63 - row:63 - row + R],
                        prods[c][:, BLOCK * j:BLOCK * (j + 1)],
                        start=(chain_cnt[ch] == 0),
                        stop=(chain_cnt[ch] == mm_per_chain - 1))
                    chain_cnt[ch] += 1
                    i_mm += 1

    if psum_chains == 1:
        d_fin = d_psums[0]
    else:
        d_fin = stat_pool.tile([R, BLOCK], f32, tag="dfin")
        nc.scalar.copy(d_fin[:], d_psums[0][:])
        for i in range(1, psum_chains):
            nc.vector.tensor_add(d_fin[:], d_fin[:], d_psums[i][:])
    exp_sb = stat_pool.tile([R, BLOCK], f32)
    nc.scalar.activation(exp_sb[:], d_fin[:],
                         mybir.ActivationFunctionType.Exp)

    # mask 0 is host-combined: -m1/(B*P) - m2/(B*Ng), so col 0 is the
    # whole linear term of the loss; mask 1 is unused on device
    rcat = stat_pool.tile([R, 4], f32)
    nc.vector.memset(rcat[:, 1:2], 0.0)
    srcs = {0: d_fin, 2: exp_sb, 3: exp_sb}
    for f in (0, 2, 3):
        scratch = stat_pool.tile([R, BLOCK], f32, tag="scr")
        nc.vector.tensor_mul(scratch[:], srcs[f][:], m_tiles[f][:])
        nc.vector.reduce_sum(rcat[:, f:f + 1], scratch[:],
                             axis=mybir.AxisListType.X)

    f_psum = psum_pool.tile([1, 4], f32, tag="final")
    nc.tensor.matmul(f_psum[:], ones_f32[0:R, :], rcat[:], start=True, stop=True)
    res = stat_pool.tile([1, 4], f32)
    nc.scalar.copy(res[:], f_psum[:])
    nc.sync.dma_start(out, res[:])


_CACHE = {}


def _compiled(S, nb, dt_name, prog_opts=None):
    key = (S, nb, dt_name, repr(sorted((prog_opts or {}).items(),
                                       key=lambda kv: kv[0])))
    if key in _CACHE:
        return _CACHE[key]
    from contextlib import ExitStack

    import concourse.bacc as bacc
    import concourse.mybir as mybir
    import concourse.tile as tile

    dt_in = getattr(mybir.dt, dt_name)
    nc = bacc.Bacc("TRN2", target_bir_lowering=False, debug=False,
                   num_devices=N_CORES)
    with tile.TileContext(nc) as tc:
        with ExitStack() as ctx:
            trace_program(nc, tc, ctx, S, nb, dt_in, **(prog_opts or {}))
    nc.compile()
    _CACHE[key] = nc
    return nc


# -------------------------------------------------------------------- kernel

def kernel(x, y, _dt_name="float16", _run_opts=None, _prog_opts=None):
    x = np.asarray(x)
    y = np.asarray(y)
    assert x.shape == (B, C, 256, 256) and y.shape == (N,)

    colmap, fams, P, Ng, M = _build_walk(y)
    E = fams.shape[0]
    S = E // N_CORES
    nb = S // BLOCK
    assert nb * BLOCK * N_CORES == E and 4 * nb <= 128

    np_dt = {"float32": np.float32, "float16": np.float16}[_dt_name]
    x3 = x.reshape(B, C, N)
    xp = np.ascontiguousarray(x3[:, :, colmap], dtype=np_dt)  # [B, C, E+1]

    # masks in d-row layout: row 4k+b of core i covers edges
    # i*S + k*BLOCK + [0, BLOCK), identical for the 4 b rows
    fams_c = fams.reshape(N_CORES, nb, 1, BLOCK)
    m = np.empty((N_CORES, 4, 4 * nb, BLOCK), dtype=np.float32)
    for f in range(4):
        m[:, f] = np.broadcast_to(fams_c == f + 1,
                                  (N_CORES, nb, 4, BLOCK)
                                  ).reshape(N_CORES, 4 * nb, BLOCK)
    # fold the two linear-family reductions into one device pass: col 0 of
    # the device result becomes the full linear term of the loss
    m[:, 0] = -m[:, 0] / (B * P) - m[:, 1] / (B * Ng)

    def aligned_copy(a, align=1 << 21):
        buf = np.empty(a.nbytes + align, dtype=np.uint8)
        off = (-buf.ctypes.data) % align
        v = buf[off:off + a.nbytes].view(a.dtype).reshape(a.shape)
        v[...] = a
        return v

    in_maps = [
        {"xp": aligned_copy(xp[:, :, i * S:(i + 1) * S + 1]),
         "msk": m[i]}
        for i in range(N_CORES)
    ]

    nc = _compiled(S, nb, _dt_name, _prog_opts)
    from concourse.bass_utils import run_bass_kernel_spmd

    res = run_bass_kernel_spmd(nc, in_maps, list(range(N_CORES)),
                               **(_run_opts or {}))
    partials = np.stack([r["out"][0] for r in res.results])  # [N_CORES, 4]
    lin, _, s3, s4 = partials.sum(axis=0, dtype=np.float64)

    n = float(B * M)
    loss = (lin
            + math.log(s3) - math.log(n) + math.log(s4) - math.log(n))
    assert np.isfinite(loss)
    out = np.float32(loss)
    if _run_opts:
        return out, res
    return out



# revision 14
# speedup vs baseline: 1.3749x; 1.0016x over previous
"""Trainium2 Bass kernel for nn_ContrastiveLoss (B=4, C=256, H=W=256).

Strategy
--------
The reference computes four families of per-position channel dot products
over columns of x viewed as [B, C, N] (N = H*W), then scalar reductions:

  fam1 (pos_sim): dot(x[:,:,pos[t]],  x[:,:,pos[t+P]])   t in [0,P)
  fam2 (neg_sim): dot(x[:,:,neg[t]],  x[:,:,neg[t+Ng]])  t in [0,Ng)
  fam3 (pn1):     dot(x[:,:,pos[t]],  x[:,:,neg[t]])     t in [0,M)
  fam4 (pn2):     dot(x[:,:,pos[t]],  x[:,:,neg[t]])     t in [M,2M)

Each column of x participates in at most two dot products, so the union of
the four pairings is a degree-<=2 graph = disjoint paths and even cycles.
The host walks those paths/cycles and emits columns in walk order; in the
permuted tensor xp every dot product is between ADJACENT columns.  The
device streams xp once (~17 MB/core fp16, the HBM roofline ~47us), computes
shifted products xp[:,:,i]*xp[:,:,i+1], reduces over C via a ones-staircase
matmul on the tensor engine (PSUM-accumulated over the two 128-channel
chunks), and applies per-family 0/1 masks to form the four partial
reductions.  Cycles are closed by re-emitting their first column; junk
edges between components and in padding have all-zero masks.

Engine assignment (measured on HW, not theory):
 * ALL products on the DVE in fp16 2x mode (~1.2us per [128,4096] tile).
   Offloading any multiplies to GpSimd is a large net loss: a concurrent
   GpSimd tensor op knocks the DVE from 1224ns to 4616ns per tile via SBUF
   port contention.
 * Staircase matmuls run ~380-400ns (not the 213ns ideal): the PE p-state
   only ramps during long uninterrupted busy streaks.  LDWEIGHTS (~110ns)
   hides under the preceding matmul.  16 warmup matmuls on junk data keep
   the PE busy while the first DMA tiles land.  Consecutive matmuls
   alternate between two PSUM accumulation chains (psum_chains=2) to avoid
   same-bank accumulate turnaround; the chains are summed at the end.
 * Chunk-combining products with a DVE add (to halve matmul count) was
   tried and is a net loss: the add serializes behind both muls and starves
   the PE.
 * Host input buffers are copied to 2 MB-aligned allocations: unaligned
   fresh allocations flip the device DRAM placement between a ~68us and a
   ~75us mode run-to-run; aligned buffers land in the fast mode.
 * Per-core HW time ~68us, vs 93.5us for the GpSimd-offload baseline.

Sharding: the edge list is split into 8 equal contiguous chunks of the
column walk (one per NeuronCore, overlapping by one column).  Each core
returns 4 partial scalars (sum d*m1, sum d*m2, sum exp(d)*m3, sum
exp(d)*m4); the host combines them into the final loss.  exp() needs no
max-subtraction: |d| < ~0.5 for this data regime, so sum(exp(d)) is stable
in fp32 (guarded by an assert on the host).
"""

import math
import sys

import numpy as np

if "/opt/trn_rl_repo" not in sys.path:  # harness runs from a fresh dir
    sys.path.insert(0, "/opt/trn_rl_repo")

B, C, N = 4, 256, 65536
N_CORES = 8
BLOCK = 512          # edges per PSUM block (= max fp32 matmul free dim)
CHUNKS = C // 128    # channel chunks of 128 partitions


# ---------------------------------------------------------------- host prep

def _build_walk(y):
    """Column permutation + per-edge family labels (0 = junk/padding)."""
    y = np.asarray(y).reshape(-1)
    pos_idx = np.nonzero(y == 1)[0]
    neg_idx = np.nonzero(y == 0)[0]
    P = pos_idx.shape[0] // 2
    Ng = neg_idx.shape[0] // 2
    M = min(P, Ng)

    nP, nN = 2 * P, 2 * Ng
    V = nP + nN
    t_pos = np.arange(nP)
    t_neg = np.arange(nN)
    nbrA = np.empty(V, dtype=np.int64)
    nbrA[:nP] = np.where(t_pos < P, t_pos + P, t_pos - P)
    nbrA[nP:] = nP + np.where(t_neg < Ng, t_neg + Ng, t_neg - Ng)
    famA = np.empty(V, dtype=np.int8)
    famA[:nP] = 1
    famA[nP:] = 2
    nbrB = np.full(V, -1, dtype=np.int64)
    nbrB[:2 * M] = nP + t_pos[:2 * M]
    nbrB[nP:nP + 2 * M] = t_neg[:2 * M]
    famB = np.zeros(V, dtype=np.int8)
    famB[:M] = 3
    famB[M:2 * M] = 4
    famB[nP:nP + M] = 3
    famB[nP + M:nP + 2 * M] = 4

    visited = np.zeros(V, dtype=bool)
    order = np.empty(V + V // 4 + 16, dtype=np.int64)
    fams_l = np.empty(order.shape[0], dtype=np.int8)
    no = 0
    ne = 0

    def walk_from(v0, is_cycle):
        nonlocal no, ne
        if no > 0:
            fams_l[ne] = 0  # junk edge joining the previous component
            ne += 1
        v = v0
        use_A = True  # endpoints/cycle starts leave via their A edge
        order[no] = v
        no += 1
        visited[v] = True
        while True:
            if use_A:
                nxt, fam = nbrA[v], famA[v]
            else:
                nxt = nbrB[v]
                if nxt < 0:
                    return
                fam = famB[v]
            if visited[nxt]:
                if is_cycle and nxt == v0 and not use_A:
                    fams_l[ne] = fam
                    ne += 1
                    order[no] = v0  # close the cycle
                    no += 1
                return
            fams_l[ne] = fam
            ne += 1
            order[no] = nxt
            no += 1
            visited[nxt] = True
            v = nxt
            use_A = not use_A

    for v0 in np.nonzero(nbrB < 0)[0]:
        if not visited[v0]:
            walk_from(int(v0), is_cycle=False)
    for v0 in range(V):
        if not visited[v0]:
            walk_from(int(v0), is_cycle=True)

    n_real = int((fams_l[:ne] > 0).sum())
    assert n_real == P + Ng + 2 * M, (n_real, P + Ng + 2 * M)

    per = N_CORES * BLOCK
    E_pad = ((ne + per - 1) // per) * per
    V_pad = E_pad + 1
    fams = np.zeros(E_pad, dtype=np.int8)
    fams[:ne] = fams_l[:ne]
    vert = np.zeros(V_pad, dtype=np.int64)
    vert[:no] = order[:no]
    colmap = np.where(vert < nP, pos_idx[np.minimum(vert, nP - 1)],
                      neg_idx[np.maximum(vert - nP, 0)])
    return colmap, fams, P, Ng, M


# ------------------------------------------------------------- device program

def trace_program(nc, tc, ctx, S, nb, dt_in, **prog_opts):
    """Emit the per-core program. S = edges/core, nb = S//BLOCK.

    DRAM tensors (per core): xp [B, C, S+1] dt_in, msk [4, 4*nb, BLOCK] f32,
    out [1, 4] f32 = (sum d*m1, sum d*m2, sum exp(d)*m3, sum exp(d)*m4).
    """
    import concourse.mybir as mybir

    f32 = mybir.dt.float32
    R = 4 * nb
    xp = nc.dram_tensor("xp", [B, C, S + 1], dt_in, kind="ExternalInput").ap()
    msk = nc.dram_tensor("msk", [4, R, BLOCK], f32, kind="ExternalInput").ap()
    out = nc.dram_tensor("out", [1, 4], f32, kind="ExternalOutput").ap()
    trace_program_aps(nc, tc, ctx, S, nb, dt_in, xp, msk, out, **prog_opts)


def trace_program_aps(nc, tc, ctx, S, nb, dt_in, xp, msk, out,
                      gpsimd_on=(), xp_bufs=12, prod_bufs=10,
                      kb_blocks=8, psum_chains=2, warmup=16):
    import concourse.mybir as mybir

    f32 = mybir.dt.float32
    R = 4 * nb  # d rows: block k, batch b -> row 4k+b

    const_pool = ctx.enter_context(tc.tile_pool(name="const", bufs=1))
    mask_pool = ctx.enter_context(tc.tile_pool(name="masks", bufs=1))
    xp_pool = ctx.enter_context(tc.tile_pool(name="xp", bufs=xp_bufs))
    prod_pool = ctx.enter_context(tc.tile_pool(name="prod", bufs=prod_bufs))
    stat_pool = ctx.enter_context(tc.tile_pool(name="stat", bufs=1))
    psum_pool = ctx.enter_context(tc.tile_pool(name="psum", bufs=1, space="PSUM"))

    # Staircase selector: zo[:, 63] = 1, else 0.  lhsT = zo[:, 63-r : 127-r]
    # is a [128, R] one-hot-column matrix that routes a partition-dim
    # column-sum into PSUM row r (other rows accumulate exact zeros) --
    # matmul PSUM outputs must start at partition 0/32/64, so rows can't be
    # addressed via the output AP.
    dt_prod = dt_in  # DVE converts on write for free; PE runs 16-bit at rate
    junk = const_pool.tile([128, BLOCK], dt_prod)
    nc.vector.memset(junk[:], 0.0)
    zo = const_pool.tile([128, 63 + R], dt_prod)
    nc.vector.memset(zo[:], 0.0)
    nc.vector.memset(zo[:, 63:64], 1.0)
    ones_f32 = const_pool.tile([128, 1], f32)
    nc.vector.memset(ones_f32[:], 1.0)

    if warmup:
        jp = psum_pool.tile([32, BLOCK], mybir.dt.float32, tag="junkp",
                            name="junk_psum")
        for _ in range(warmup):
            # keeps the PE p-state ramped while real products are not ready
            nc.tensor.matmul(jp[:, :], junk[:, 0:32], junk[:, 0:BLOCK],
                             start=True, stop=True, skip_group_check=True)
    m_tiles = []
    for f in range(4):
        mt = mask_pool.tile([R, BLOCK], f32, tag=f"m{f}")
        nc.sync.dma_start(mt[:], msk[f])
        m_tiles.append(mt)

    d_psums = [psum_pool.tile([R, BLOCK], f32, tag=f"d{i}", name=f"d_psum{i}")
               for i in range(psum_chains)]

    # KB 512-edge blocks per DMA/mul tile: fewer, larger DMAs and DVE ops
    KB = kb_blocks or (4 if nb % 4 == 0 else (2 if nb % 2 == 0 else 1))
    assert nb % KB == 0
    W = KB * BLOCK
    n_mm = nb * B * CHUNKS
    mm_per_chain = n_mm // psum_chains
    assert n_mm % psum_chains == 0
    chain_cnt = [0] * psum_chains
    i_mm = 0
    mul_i = 0
    for kb in range(nb // KB):
        for b in range(B):
            prods = []
            for c in range(CHUNKS):
                t = xp_pool.tile([128, W + 1], dt_in)
                nc.sync.dma_start(
                    t[:], xp[b, 128 * c:128 * (c + 1), W * kb:W * (kb + 1) + 1])
                p = prod_pool.tile([128, W], dt_prod)
                # DVE is the mul bottleneck; GpSimd runs these ~1.7x
                # slower, so balance ~3/8 of them onto it
                eng = nc.gpsimd if (gpsimd_on and
                                    mul_i % 8 in gpsimd_on) else nc.vector
                eng.tensor_mul(p[:], t[:, 0:W], t[:, 1:W + 1])
                mul_i += 1
                prods.append(p)
            for j in range(KB):
                row = 4 * (kb * KB + j) + b
                for c in range(CHUNKS):
                    ch = i_mm % psum_chains
                    nc.tensor.matmul(
                        d_psums[ch][:, :], zo[:, 63 - row:63 - row + R],
                        prods[c][:, BLOCK * j:BLOCK * (j + 1)],
                        start=(chain_cnt[ch] == 0),
                        stop=(chain_cnt[ch] == mm_per_chain - 1))
                    chain_cnt[ch] += 1
                    i_mm += 1

    if psum_chains == 1:
        d_fin = d_psums[0]
    else:
        d_fin = stat_pool.tile([R, BLOCK], f32, tag="dfin")
        nc.scalar.copy(d_fin[:], d_psums[0][:])
        for i in range(1, psum_chains):
            nc.vector.tensor_add(d_fin[:], d_fin[:], d_psums[i][:])
    exp_sb = stat_pool.tile([R, BLOCK], f32)
    nc.scalar.activation(exp_sb[:], d_fin[:],
                         mybir.ActivationFunctionType.Exp)

    # mask 0 is host-combined: -m1/(B*P) - m2/(B*Ng), so col 0 is the
    # whole linear term of the loss; mask 1 is unused on device
    rcat = stat_pool.tile([R, 4], f32)
    nc.vector.memset(rcat[:, 1:2], 0.0)
    srcs = {0: d_fin, 2: exp_sb, 3: exp_sb}
    for f in (0, 2, 3):
        scratch = stat_pool.tile([R, BLOCK], f32, tag="scr")
        nc.vector.tensor_mul(scratch[:], srcs[f][:], m_tiles[f][:])
        nc.vector.reduce_sum(rcat[:, f:f + 1], scratch[:],
                             axis=mybir.AxisListType.X)

    f_psum = psum_pool.tile([1, 4], f32, tag="final")
    nc.tensor.matmul(f_psum[:], ones_f32[0:R, :], rcat[:], start=True, stop=True)
    res = stat_pool.tile([1, 4], f32)
    nc.scalar.copy(res[:], f_psum[:])
    nc.sync.dma_start(out, res[:])


_CACHE = {}


def _compiled(S, nb, dt_name, prog_opts=None):
    key = (S, nb, dt_name, repr(sorted((prog_opts or {}).items(),
                                       key=lambda kv: kv[0])))
    if key in _CACHE:
        return _CACHE[key]
    from contextlib import ExitStack

    import concourse.bacc as bacc
    import concourse.mybir as mybir
    import concourse.tile as tile

    dt_in = getattr(mybir.dt, dt_name)
    nc = bacc.Bacc("TRN2", target_bir_lowering=False, debug=False,
                   num_devices=N_CORES)
    with tile.TileContext(nc) as tc:
        with ExitStack() as ctx:
            trace_program(nc, tc, ctx, S, nb, dt_in, **(prog_opts or {}))
    nc.compile()
    _CACHE[key] = nc
    return nc


# -------------------------------------------------------------------- kernel

def kernel(x, y, _dt_name="float16", _run_opts=None, _prog_opts=None):
    x = np.asarray(x)
    y = np.asarray(y)
    assert x.shape == (B, C, 256, 256) and y.shape == (N,)

    colmap, fams, P, Ng, M = _build_walk(y)
    E = fams.shape[0]
    S = E // N_CORES
    nb = S // BLOCK
    assert nb * BLOCK * N_CORES == E and 4 * nb <= 128

    np_dt = {"float32": np.float32, "float16": np.float16}[_dt_name]
    x3 = x.reshape(B, C, N)
    xp = np.ascontiguousarray(x3[:, :, colmap], dtype=np_dt)  # [B, C, E+1]

    # masks in d-row layout: row 4k+b of core i covers edges
    # i*S + k*BLOCK + [0, BLOCK), identical for the 4 b rows
    fams_c = fams.reshape(N_CORES, nb, 1, BLOCK)
    m = np.empty((N_CORES, 4, 4 * nb, BLOCK), dtype=np.float32)
    for f in range(4):
        m[:, f] = np.broadcast_to(fams_c == f + 1,
                                  (N_CORES, nb, 4, BLOCK)
                                  ).reshape(N_CORES, 4 * nb, BLOCK)
    # fold the two linear-family reductions into one device pass: col 0 of
    # the device result becomes the full linear term of the loss
    m[:, 0] = -m[:, 0] / (B * P) - m[:, 1] / (B * Ng)

    def aligned_copy(a, align=1 << 21):
        buf = np.empty(a.nbytes + align, dtype=np.uint8)
        off = (-buf.ctypes.data) % align
        v = buf[off:off + a.nbytes].view(a.dtype).reshape(a.shape)
        v[...] = a
        return v

    in_maps = [
        {"xp": aligned_copy(xp[:, :, i * S:(i + 1) * S + 1]),
         "msk": m[i]}
        for i in range(N_CORES)
    ]

    nc = _compiled(S, nb, _dt_name, _prog_opts)
    from concourse.bass_utils import run_bass_kernel_spmd

    res = run_bass_kernel_spmd(nc, in_maps, list(range(N_CORES)),
                               **(_run_opts or {}))
    partials = np.stack([r["out"][0] for r in res.results])  # [N_CORES, 4]
    lin, _, s3, s4 = partials.sum(axis=0, dtype=np.float64)

    n = float(B * M)
    loss = (lin
            + math.log(s3) - math.log(n) + math.log(s4) - math.log(n))
    assert np.isfinite(loss)
    out = np.float32(loss)
    if _run_opts:
        return out, res
    return out

